# revision 31
# baseline (speedup 1.0000x reference)
"""GQA attention kernel for Trainium2 (Bass/Tile), 8-core SPMD.

Problem: B=2, N=2048, DIM=1024, 16 query heads / 4 KV heads, head_dim=64, fp32.
Sharding: core c = (batch b=c//4, kv-group g=c%4). Each core computes its
group's 4 query heads + 1 shared KV head over the full sequence, and a partial
output projection (its 256 rows of Wo). Host sums the 4 group partials per
batch and adds the bias.

Layout per core:
  xT    [128, 8, N] f32r : x^T (PE transposes with an f32r identity)
  qt    [128, 2, N] f32r : Q^T head pairs (head 2p on partitions 0-63, 2p+1 on
                           64-127)
  kkT   [128, N]    f32r : K^T duplicated across partition halves (DMA dup)
  vn    [128, 16, 65] bf16: V in normal layout (keys on partitions) + ones col
  aoutT [128, 2, N] f32r : normalized attention out^T for the out-projection

Scores are computed transposed (S^T [128 keys, 512 queries]); exp on Act; P@V
uses P^T tiles as the *stationary* operand and V as the moving operand,
producing [queries, 64] in PSUM at 64 rows/matmul instead of 128; sum-of-exp
rides on 1-row ones-matmuls into a dedicated PSUM bank.

PSUM budget (8 banks): scores 2x[128,1024] double-buffered (4) + P@V
accumulators 2x[128,512] (2, two heads per bank) + transpose staging (1) +
sum-of-exp (1). Projection matmuls share the score pool, interleaved
fine-grained between score tiles so no engine convoys behind one pool.
"""

import sys

if "/opt/trn_rl_repo" not in sys.path:
    sys.path.insert(0, "/opt/trn_rl_repo")

from collections import deque
from contextlib import ExitStack

import ml_dtypes
import numpy as np

BF16_NP = ml_dtypes.bfloat16

import concourse.bass as bass
import concourse.mybir as mybir
import concourse.tile as tile
from concourse import bacc, bass_utils
from concourse.bass import ds, ts
from concourse.masks import make_identity

F32 = mybir.dt.float32
F32R = mybir.dt.float32r
BF16 = mybir.dt.bfloat16
FP8 = mybir.dt.float8e4
DROW = mybir.MatmulPerfMode.DoubleRow
EXPF = mybir.ActivationFunctionType.Exp

DIM = 1024
D = 64  # head dim
SCALE = D ** -0.5


def build_nc(NSEQ=2048):
    KT = NSEQ // 128   # key tiles
    QC = NSEQ // 512   # query chunks of 512
    DKT = DIM // 128   # contraction tiles for projections

    nc = bacc.Bacc("TRN2", target_bir_lowering=False, debug=False)
    x = nc.dram_tensor("x", [NSEQ, DIM], F32, kind="ExternalInput").ap()
    wq = nc.dram_tensor("wq", [DIM, 256], BF16, kind="ExternalInput").ap()
    wkv = nc.dram_tensor("wkv", [DIM, 128], BF16, kind="ExternalInput").ap()
    wo = nc.dram_tensor("wo", [256, DIM], F32, kind="ExternalInput").ap()
    out = nc.dram_tensor("out", [DIM, NSEQ], F32, kind="ExternalOutput").ap()

    with tile.TileContext(nc) as tc, ExitStack() as ctx:
        sb = ctx.enter_context(tc.tile_pool(name="sb", bufs=1))

        wq_sb = sb.tile([128, DKT, 256], BF16)
        wkv_sb = sb.tile([128, DKT, 128], BF16)
        wo_sb = sb.tile([128, 2, DIM], F32R)
        ident = sb.tile([128, 128], F32)
        identr = sb.tile([128, 128], F32R)
        identb = sb.tile([128, 128], BF16)
        warm_in = sb.tile([128, 1], F32)
        warm = sb.tile([128, 1], F32)

        make_identity(nc, ident)
        nc.vector.tensor_copy(identr, ident)
        nc.vector.tensor_copy(identb, ident)
        nc.vector.memset(warm_in, 1.0)
        # preload the exp table set off the critical path
        nc.scalar.activation(out=warm, in_=warm_in, func=EXPF, scale=1.0)

        xT = sb.tile([128, DKT, NSEQ], BF16)
        qbT = sb.tile([128, 2, NSEQ], BF16)
        kbT = sb.tile([128, NSEQ], BF16)
        vn = sb.tile([128, KT, D + 1], BF16)
        aoutT = sb.tile([128, 2, NSEQ], F32R)
        nc.vector.memset(vn, 1.0)

        xpool = ctx.enter_context(tc.tile_pool(name="xp", bufs=3))
        xbp = ctx.enter_context(tc.tile_pool(name="xb", bufs=3))
        vtp = ctx.enter_context(tc.tile_pool(name="vtp", bufs=2))
        ptp = ctx.enter_context(tc.tile_pool(name="ptp", bufs=12))
        rrp = ctx.enter_context(tc.tile_pool(name="rrp", bufs=2))
        aop = ctx.enter_context(tc.tile_pool(name="aop", bufs=2))
        outp = ctx.enter_context(tc.tile_pool(name="outp", bufs=4))
        # PSUM: ps_sc 2x[128,1024] (banks 0-3), ps_pv 2x[128,512] (4-5),
        # ps_pj 1x[128,512] (6), ps_su 1x[128,16] (7)
        ps_sc = ctx.enter_context(tc.tile_pool(name="ps_sc", bufs=2, space="PSUM"))
        ps_pv = ctx.enter_context(tc.tile_pool(name="ps_pv", bufs=2, space="PSUM"))
        ps_pj = ctx.enter_context(tc.tile_pool(name="ps_pj", bufs=1, space="PSUM"))
        ps_su = ctx.enter_context(tc.tile_pool(name="ps_su", bufs=1, space="PSUM"))

        state = {}     # qc -> [hp0_tile, hp1_tile] each [128, 2, 4, 64] view
        sums = {}      # qc -> [128, 16] psum tile (cols h*4+i)
        pending_pv = []
        fillq = deque()

        def fill(n=1):
            for _ in range(n):
                if not fillq:
                    return
                fillq.popleft()()

        # ---------------- work units ----------------
        def unit_ptrq(sg):
            """Transpose x DIM-tiles 0-3 of chunk sg into one 2-bank bf16 psum."""
            def run():
                ptr = ps_sc.tile([128, 2048], BF16, tag="sc", name=f"ptrq{sg}")
                for d in range(4):
                    for i in range(4):
                        nc.tensor.transpose(ptr[:, ds(d * 512 + i * 128, 128)],
                                            xs_tiles[sg][:, i, ts(d, 128)], identb)
                nc.vector.tensor_copy(xT[:, ds(0, 4), ds(sg * 512, 512)], ptr)
            return run

        def unit_ptrp(sg, dp):
            """Transpose x DIM-tiles 2dp..2dp+1 into one 1-bank bf16 psum."""
            def run():
                ptr = ps_pj.tile([128, 1024], BF16, tag="pj", name=f"ptrp{sg}_{dp}")
                for k in range(2):
                    d = 2 * dp + k
                    for i in range(4):
                        nc.tensor.transpose(ptr[:, ds(k * 512 + i * 128, 128)],
                                            xs_tiles[sg][:, i, ts(d, 128)], identb)
                nc.vector.tensor_copy(xT[:, ds(2 * dp, 2), ds(sg * 512, 512)], ptr)
            return run

        def unit_pkv(sg):
            def run():
                pkv = ps_pj.tile([128, 512], F32, tag="pj", name=f"pkv{sg}")
                for d in range(DKT):
                    nc.tensor.matmul(pkv[:, 0:512], wkv_sb[:, d, :],
                                     xT[:, d, ds(sg * 512, 512)],
                                     start=(d == 0), stop=(d == DKT - 1))
                nc.vector.tensor_copy(kbT[ds(0, 64), ds(sg * 512, 512)],
                                      pkv[ds(0, 64), 0:512])
                nc.sync.dma_start(out=kbT[ds(64, 64), ds(sg * 512, 512)],
                                  in_=kbT[ds(0, 64), ds(sg * 512, 512)])
                vtmp = vtp.tile([64, 512], BF16, tag="vt", name=f"vt{sg}")
                nc.vector.tensor_copy(vtmp, pkv[ds(64, 64), 0:512])
                vtmp_tiles[sg] = vtmp
            return run

        def unit_ptv(sg):
            def run():
                ptv = ps_pj.tile([128, 1024], BF16, tag="pj", name=f"ptv{sg}")
                for i in range(4):
                    nc.tensor.transpose(ptv[:, ds(i * D, D)], vtmp_tiles[sg][:, ts(i, 128)],
                                        identb[0:64, 0:64])
                nc.vector.tensor_copy(vn[:, ds(sg * 4, 4), 0:D], ptv[:, 0:4 * D])
            return run

        def unit_qt(qc, p, hlf=None):
            def run():
                if hlf is None:
                    qw, off = 512, 0
                else:
                    qw, off = 256, hlf * 256
                pq = ps_sc.tile([128, 1024], F32, tag="sc", name=f"pq{qc}_{p}_{off}")
                for d in range(DKT):
                    nc.tensor.matmul(pq[:, 0:qw], wq_sb[:, d, ts(p, 128)],
                                     xT[:, d, ds(qc * 512 + off, qw)],
                                     start=(d == 0), stop=(d == DKT - 1))
                nc.vector.tensor_copy(qbT[:, p, ds(qc * 512 + off, qw)], pq[:, 0:qw])
            return run

        def unit_po(qc, od, pool=None, use_act=False):
            def run():
                p_ = pool if pool is not None else ps_sc
                tag = "pj" if p_ is ps_pj else "sc"
                shape = [128, 512] if p_ is ps_pj else [128, 1024]
                po = p_.tile(shape, F32, tag=tag, name=f"po{qc}_{od}")
                nc.tensor.matmul(po[:, 0:512], wo_sb[:, 0, ts(od, 128)],
                                 aoutT[:, 0, ds(qc * 512, 512)], start=True, stop=False)
                nc.tensor.matmul(po[:, 0:512], wo_sb[:, 1, ts(od, 128)],
                                 aoutT[:, 1, ds(qc * 512, 512)], start=False, stop=True)
                ot = outp.tile([128, 512], F32, tag="ot", name=f"ot{qc}_{od}")
                if use_act and od % 2 == 1:
                    nc.scalar.activation(out=ot, in_=po[:, 0:512],
                                         func=mybir.ActivationFunctionType.Copy,
                                         scale=1.0)
                else:
                    nc.vector.tensor_copy(ot, po[:, 0:512])
                nc.sync.dma_start(out=out[ts(od, 128), ds(qc * 512, 512)], in_=ot)
            return run

        # ---------------- attention ----------------
        def flush_pv():
            for (qc_, j_, h_, pt_) in pending_pv:
                hp, hh = h_ // 2, h_ % 2
                for t in range(2):
                    kt = 2 * j_ + t
                    for i in range(4):
                        stn = pt_[:, ds(t * 512 + i * 128, 128)]
                        # start=True zeroes the whole 2KB PSUM bank: only the
                        # first series touching each bank may set it.
                        nc.tensor.matmul(state[qc_][hp][:, hh, i, :], stn,
                                         vn[:, kt, 0:D],
                                         start=(kt == 0 and i == 0 and hh == 0),
                                         stop=(kt == KT - 1),
                                         skip_group_check=True)
                        nc.tensor.matmul(sums[qc_][:, ds(h_ * 4 + i, 1)], stn,
                                         vn[:, kt, D:D + 1],
                                         start=(kt == 0 and i == 0 and h_ == 0),
                                         stop=(kt == KT - 1),
                                         skip_group_check=True)
            pending_pv.clear()

        def emit_quanta(qc, j, mid_fills=(2,)):
            new_pv = []
            for h in range(4):
                p, i = h // 2, h % 2
                psc = ps_sc.tile([128, 1024], F32, tag="sc", name=f"psc{qc}_{j}_{h}")
                for t in range(2):
                    kt = 2 * j + t
                    nc.tensor.matmul(psc[:, ds(t * 512, 512)],
                                     kbT[ds(i * 64, 64), ts(kt, 128)],
                                     qbT[ds(i * 64, 64), p, ds(qc * 512, 512)],
                                     start=True, stop=True)
                pt = ptp.tile([128, 1024], BF16, tag="pt", name=f"pt{qc}_{j}_{h}")
                nc.scalar.activation(out=pt, in_=psc, func=EXPF, scale=SCALE)
                new_pv.append((qc, j, h, pt))
                if h == 1:
                    flush_pv()
                if h in mid_fills:
                    fill(1)
            pending_pv.extend(new_pv)

        def alloc_state(qc):
            state[qc] = [
                ps_pv.tile([128, 2, 4, D], F32, tag="pv", name=f"pv{qc}_{hp}")
                for hp in range(2)
            ]
            sums[qc] = ps_su.tile([128, 16], F32, tag="su", name=f"su{qc}")

        def emit_norm(qc, use_act=False):
            COPYF = mybir.ActivationFunctionType.Copy
            rr = rrp.tile([128, 16], F32, tag="rr", name=f"rr{qc}")
            nc.vector.reciprocal(out=rr, in_=sums[qc])
            ao = aop.tile([128, 4, 4, D], F32R, tag="ao", name=f"ao{qc}")
            for hp in range(2):
                for hh in range(2):
                    h = 2 * hp + hh
                    for i in range(4):
                        if use_act and (i % 2 == 1):
                            nc.scalar.activation(out=ao[:, i, h, :],
                                                 in_=state[qc][hp][:, hh, i, :],
                                                 func=COPYF,
                                                 scale=rr[:, ds(h * 4 + i, 1)])
                        else:
                            nc.vector.tensor_scalar_mul(ao[:, i, h, :],
                                                        state[qc][hp][:, hh, i, :],
                                                        rr[:, ds(h * 4 + i, 1)])
                pat = ps_pj.tile([128, 512], F32R, tag="pj", name=f"pat{qc}_{hp}")
                for i in range(4):
                    nc.tensor.transpose(pat[:, ds(i * 128, 128)],
                                        ao[:, i, ds(2 * hp, 2), :], identr)
                if use_act and hp == 1:
                    nc.scalar.activation(out=aoutT[:, hp, ds(qc * 512, 512)],
                                         in_=pat, func=COPYF, scale=1.0)
                else:
                    nc.vector.tensor_copy(aoutT[:, hp, ds(qc * 512, 512)], pat)

        # ---------------- schedule ----------------
        xs_tiles = {}
        vtmp_tiles = {}

        def dma_x(sg):
            if sg in xs_tiles:
                return
            xs = xpool.tile([128, 4, DIM], F32, tag="xs", name=f"xs{sg}")
            for hlf in range(2):
                src_ap = x[ds(sg * 512 + hlf * 256, 256), :].rearrange(
                    "(i p) m -> p i m", p=128)
                nc.sync.dma_start(out=xs[:, ds(hlf * 2, 2), :], in_=src_ap)
            xb = xbp.tile([128, 4, DIM], BF16, tag="xb", name=f"xb{sg}")
            COPYF = mybir.ActivationFunctionType.Copy
            for i in range(4):
                for hlf in range(2):
                    dst = xb[:, i, ds(hlf * 512, 512)]
                    srca = xs[:, i, ds(hlf * 512, 512)]
                    if sg < 2 and hlf == 0:
                        nc.scalar.activation(out=dst, in_=srca, func=COPYF, scale=1.0)
                    elif sg >= 2 and hlf == 1:
                        nc.vector.tensor_copy(dst, srca)
                    else:
                        nc.gpsimd.tensor_copy(dst, srca)
            xs_tiles[sg] = xb

        def sgroup_units(sg, eager):
            dma_x(sg)
            units = [unit_ptrq(sg), unit_ptrp(sg, 2), unit_ptrp(sg, 3),
                     unit_pkv(sg), unit_ptv(sg)]
            if eager:
                for u in units:
                    u()
            else:
                fillq.extend(units)

        # prologue: x chunk 0 first on the DMA engines, then weights
        dma_x(0)
        nc.sync.dma_start(out=wkv_sb, in_=wkv.rearrange("(t p) m -> p t m", p=128))
        nc.sync.dma_start(out=wq_sb, in_=wq.rearrange("(t p) m -> p t m", p=128))
        sgroup_units(0, eager=True)
        unit_qt(0, 0, 0)()
        unit_qt(0, 0, 1)()
        unit_qt(0, 1, 0)()
        unit_qt(0, 1, 1)()
        nc.sync.dma_start(out=wo_sb, in_=wo.rearrange("(t p) m -> p t m", p=128).bitcast(F32R))
        alloc_state(0)
        sgroup_units(1, eager=False)
        for j in range(KT // 2):
            if j == 0:
                dma_x(2)
            if j == 1:
                sgroup_units(2, eager=False)
            if j == 2:
                dma_x(3)
            if j == 3:
                sgroup_units(3, eager=False)
            if j >= 4 and j <= 7:
                fillq.append(unit_qt(1, (j - 4) // 2, (j - 4) % 2))
            emit_quanta(0, j, mid_fills=(0, 1, 2, 3))
            fill(1)
        for qc in range(1, QC):
            flush_pv()               # (qc-1, 7) into state[qc-1] during exp drain
            emit_quanta(qc, 0)
            emit_norm(qc - 1)
            alloc_state(qc)
            for j in range(1, KT // 2):
                fillq.append(unit_po(qc - 1, j - 1))
                if j >= 4 and qc + 1 < QC:
                    fillq.append(unit_qt(qc + 1, (j - 4) // 2, (j - 4) % 2))
                if j == 7:
                    fillq.append(unit_po(qc - 1, 7))
                emit_quanta(qc, j)
                fill(1)
        flush_pv()
        emit_norm(QC - 1, use_act=True)
        fill(len(fillq))
        for od in range(8):
            pool = ps_pj if od % 3 == 2 else ps_sc
            unit_po(QC - 1, od, pool, use_act=True)()

    nc.compile()
    return nc


_CACHE = {}


def _get_nc(NSEQ):
    if NSEQ not in _CACHE:
        _CACHE[NSEQ] = build_nc(NSEQ)
    return _CACHE[NSEQ]


def kernel(x, Wq, Wk, Wv, Wo, bo):
    """Full-input entry point: shard over 8 cores, run, gather."""
    x, Wq, Wk, Wv, Wo, bo = (np.asarray(a, np.float32) for a in (x, Wq, Wk, Wv, Wo, bo))
    B, N, C = x.shape
    nc = _get_nc(N)
    in_maps = []
    for c in range(8):
        b, g = c // 4, c % 4
        in_maps.append({
            "x": np.ascontiguousarray(x[b]),
            "wq": np.ascontiguousarray(Wq[:, g * 256:(g + 1) * 256]).astype(BF16_NP),
            "wkv": np.ascontiguousarray(np.concatenate(
                [Wk[:, g * D:(g + 1) * D], Wv[:, g * D:(g + 1) * D]],
                axis=1)).astype(BF16_NP),
            "wo": np.ascontiguousarray(Wo[g * 256:(g + 1) * 256, :]),
        })
    res = bass_utils.run_bass_kernel_spmd(nc, in_maps, core_ids=list(range(8)))
    outs = [res.results[c]["out"] for c in range(8)]
    full = np.empty((B, N, C), np.float32)
    for b in range(B):
        acc = outs[4 * b].astype(np.float32)
        for g in range(1, 4):
            acc = acc + outs[4 * b + g]
        full[b] = acc.T + bo[None, :]
    return full


# revision 32
# speedup vs baseline: 1.0001x; 1.0001x over previous
"""GQA attention kernel for Trainium2 (Bass/Tile), 8-core SPMD.

Problem: B=2, N=2048, DIM=1024, 16 query heads / 4 KV heads, head_dim=64, fp32.
Sharding: core c = (batch b=c//4, kv-group g=c%4). Each core computes its
group's 4 query heads + 1 shared KV head over the full sequence, and a partial
output projection (its 256 rows of Wo). Host sums the 4 group partials per
batch and adds the bias.

Layout per core:
  xT    [128, 8, N] f32r : x^T (PE transposes with an f32r identity)
  qt    [128, 2, N] f32r : Q^T head pairs (head 2p on partitions 0-63, 2p+1 on
                           64-127)
  kkT   [128, N]    f32r : K^T duplicated across partition halves (DMA dup)
  vn    [128, 16, 65] bf16: V in normal layout (keys on partitions) + ones col
  aoutT [128, 2, N] f32r : normalized attention out^T for the out-projection

Scores are computed transposed (S^T [128 keys, 512 queries]); exp on Act; P@V
uses P^T tiles as the *stationary* operand and V as the moving operand,
producing [queries, 64] in PSUM at 64 rows/matmul instead of 128; sum-of-exp
rides on 1-row ones-matmuls into a dedicated PSUM bank.

PSUM budget (8 banks): scores 2x[128,1024] double-buffered (4) + P@V
accumulators 2x[128,512] (2, two heads per bank) + transpose staging (1) +
sum-of-exp (1). Projection matmuls share the score pool, interleaved
fine-grained between score tiles so no engine convoys behind one pool.
"""

import sys

if "/opt/trn_rl_repo" not in sys.path:
    sys.path.insert(0, "/opt/trn_rl_repo")

from collections import deque
from contextlib import ExitStack

import ml_dtypes
import numpy as np

BF16_NP = ml_dtypes.bfloat16

import concourse.bass as bass
import concourse.mybir as mybir
import concourse.tile as tile
from concourse import bacc, bass_utils
from concourse.bass import ds, ts
from concourse.masks import make_identity

F32 = mybir.dt.float32
F32R = mybir.dt.float32r
BF16 = mybir.dt.bfloat16
FP8 = mybir.dt.float8e4
DROW = mybir.MatmulPerfMode.DoubleRow
EXPF = mybir.ActivationFunctionType.Exp

DIM = 1024
D = 64  # head dim
SCALE = D ** -0.5


def build_nc(NSEQ=2048):
    KT = NSEQ // 128   # key tiles
    QC = NSEQ // 512   # query chunks of 512
    DKT = DIM // 128   # contraction tiles for projections

    nc = bacc.Bacc("TRN2", target_bir_lowering=False, debug=False)
    x = nc.dram_tensor("x", [NSEQ, DIM], F32, kind="ExternalInput").ap()
    wq = nc.dram_tensor("wq", [DIM, 256], BF16, kind="ExternalInput").ap()
    wkv = nc.dram_tensor("wkv", [DIM, 128], BF16, kind="ExternalInput").ap()
    wo = nc.dram_tensor("wo", [256, DIM], F32, kind="ExternalInput").ap()
    out = nc.dram_tensor("out", [DIM, NSEQ], F32, kind="ExternalOutput").ap()

    with tile.TileContext(nc) as tc, ExitStack() as ctx:
        sb = ctx.enter_context(tc.tile_pool(name="sb", bufs=1))

        wq_sb = sb.tile([128, DKT, 256], BF16)
        wkv_sb = sb.tile([128, DKT, 128], BF16)
        wo_sb = sb.tile([128, 2, DIM], F32R)
        ident = sb.tile([128, 128], F32)
        identr = sb.tile([128, 128], F32R)
        identb = sb.tile([128, 128], BF16)
        warm_in = sb.tile([128, 1], F32)
        warm = sb.tile([128, 1], F32)

        make_identity(nc, ident)
        nc.vector.tensor_copy(identr, ident)
        nc.vector.tensor_copy(identb, ident)
        nc.vector.memset(warm_in, 1.0)
        # preload the exp table set off the critical path
        nc.scalar.activation(out=warm, in_=warm_in, func=EXPF, scale=1.0)

        xT = sb.tile([128, DKT, NSEQ], BF16)
        qbT = sb.tile([128, 2, NSEQ], BF16)
        kbT = sb.tile([128, NSEQ], BF16)
        vn = sb.tile([128, KT, D + 1], BF16)
        aoutT = sb.tile([128, 2, NSEQ], F32R)
        nc.vector.memset(vn, 1.0)

        xpool = ctx.enter_context(tc.tile_pool(name="xp", bufs=3))
        xbp = ctx.enter_context(tc.tile_pool(name="xb", bufs=3))
        vtp = ctx.enter_context(tc.tile_pool(name="vtp", bufs=2))
        ptp = ctx.enter_context(tc.tile_pool(name="ptp", bufs=12))
        rrp = ctx.enter_context(tc.tile_pool(name="rrp", bufs=2))
        aop = ctx.enter_context(tc.tile_pool(name="aop", bufs=2))
        outp = ctx.enter_context(tc.tile_pool(name="outp", bufs=4))
        # PSUM: ps_sc 2x[128,1024] (banks 0-3), ps_pv 2x[128,512] (4-5),
        # ps_pj 1x[128,512] (6), ps_su 1x[128,16] (7)
        ps_sc = ctx.enter_context(tc.tile_pool(name="ps_sc", bufs=2, space="PSUM"))
        ps_pv = ctx.enter_context(tc.tile_pool(name="ps_pv", bufs=2, space="PSUM"))
        ps_pj = ctx.enter_context(tc.tile_pool(name="ps_pj", bufs=1, space="PSUM"))
        ps_su = ctx.enter_context(tc.tile_pool(name="ps_su", bufs=1, space="PSUM"))

        state = {}     # qc -> [hp0_tile, hp1_tile] each [128, 2, 4, 64] view
        sums = {}      # qc -> [128, 16] psum tile (cols h*4+i)
        pending_pv = []
        fillq = deque()

        def fill(n=1):
            for _ in range(n):
                if not fillq:
                    return
                fillq.popleft()()

        # ---------------- work units ----------------
        def unit_ptrq(sg):
            """Transpose x DIM-tiles 0-3 of chunk sg into one 2-bank bf16 psum."""
            def run():
                ptr = ps_sc.tile([128, 2048], BF16, tag="sc", name=f"ptrq{sg}")
                for d in range(4):
                    for i in range(4):
                        nc.tensor.transpose(ptr[:, ds(d * 512 + i * 128, 128)],
                                            xs_tiles[sg][:, i, ts(d, 128)], identb)
                nc.vector.tensor_copy(xT[:, ds(0, 4), ds(sg * 512, 512)], ptr)
            return run

        def unit_ptrp(sg, dp):
            """Transpose x DIM-tiles 2dp..2dp+1 into one 1-bank bf16 psum."""
            def run():
                ptr = ps_pj.tile([128, 1024], BF16, tag="pj", name=f"ptrp{sg}_{dp}")
                for k in range(2):
                    d = 2 * dp + k
                    for i in range(4):
                        nc.tensor.transpose(ptr[:, ds(k * 512 + i * 128, 128)],
                                            xs_tiles[sg][:, i, ts(d, 128)], identb)
                nc.vector.tensor_copy(xT[:, ds(2 * dp, 2), ds(sg * 512, 512)], ptr)
            return run

        def unit_pkv(sg):
            def run():
                pkv = ps_pj.tile([128, 512], F32, tag="pj", name=f"pkv{sg}")
                for d in range(DKT):
                    nc.tensor.matmul(pkv[:, 0:512], wkv_sb[:, d, :],
                                     xT[:, d, ds(sg * 512, 512)],
                                     start=(d == 0), stop=(d == DKT - 1))
                nc.vector.tensor_copy(kbT[ds(0, 64), ds(sg * 512, 512)],
                                      pkv[ds(0, 64), 0:512])
                nc.sync.dma_start(out=kbT[ds(64, 64), ds(sg * 512, 512)],
                                  in_=kbT[ds(0, 64), ds(sg * 512, 512)])
                vtmp = vtp.tile([64, 512], BF16, tag="vt", name=f"vt{sg}")
                nc.vector.tensor_copy(vtmp, pkv[ds(64, 64), 0:512])
                vtmp_tiles[sg] = vtmp
            return run

        def unit_ptv(sg):
            def run():
                ptv = ps_pj.tile([128, 1024], BF16, tag="pj", name=f"ptv{sg}")
                for i in range(4):
                    nc.tensor.transpose(ptv[:, ds(i * D, D)], vtmp_tiles[sg][:, ts(i, 128)],
                                        identb[0:64, 0:64])
                nc.vector.tensor_copy(vn[:, ds(sg * 4, 4), 0:D], ptv[:, 0:4 * D])
            return run

        def unit_qt(qc, p, hlf=None):
            def run():
                if hlf is None:
                    qw, off = 512, 0
                else:
                    qw, off = 256, hlf * 256
                pq = ps_sc.tile([128, 1024], F32, tag="sc", name=f"pq{qc}_{p}_{off}")
                for d in range(DKT):
                    nc.tensor.matmul(pq[:, 0:qw], wq_sb[:, d, ts(p, 128)],
                                     xT[:, d, ds(qc * 512 + off, qw)],
                                     start=(d == 0), stop=(d == DKT - 1))
                nc.vector.tensor_copy(qbT[:, p, ds(qc * 512 + off, qw)], pq[:, 0:qw])
            return run

        def unit_po(qc, od, pool=None, use_act=False):
            def run():
                p_ = pool if pool is not None else ps_sc
                tag = "pj" if p_ is ps_pj else "sc"
                shape = [128, 512] if p_ is ps_pj else [128, 1024]
                po = p_.tile(shape, F32, tag=tag, name=f"po{qc}_{od}")
                nc.tensor.matmul(po[:, 0:512], wo_sb[:, 0, ts(od, 128)],
                                 aoutT[:, 0, ds(qc * 512, 512)], start=True, stop=False)
                nc.tensor.matmul(po[:, 0:512], wo_sb[:, 1, ts(od, 128)],
                                 aoutT[:, 1, ds(qc * 512, 512)], start=False, stop=True)
                ot = outp.tile([128, 512], F32, tag="ot", name=f"ot{qc}_{od}")
                if use_act and od % 2 == 1:
                    nc.scalar.activation(out=ot, in_=po[:, 0:512],
                                         func=mybir.ActivationFunctionType.Copy,
                                         scale=1.0)
                else:
                    nc.vector.tensor_copy(ot, po[:, 0:512])
                nc.sync.dma_start(out=out[ts(od, 128), ds(qc * 512, 512)], in_=ot)
            return run

        # ---------------- attention ----------------
        def flush_pv():
            for (qc_, j_, h_, pt_) in pending_pv:
                hp, hh = h_ // 2, h_ % 2
                for t in range(2):
                    kt = 2 * j_ + t
                    for i in range(4):
                        stn = pt_[:, ds(t * 512 + i * 128, 128)]
                        # start=True zeroes the whole 2KB PSUM bank: only the
                        # first series touching each bank may set it.
                        nc.tensor.matmul(state[qc_][hp][:, hh, i, :], stn,
                                         vn[:, kt, 0:D],
                                         start=(kt == 0 and i == 0 and hh == 0),
                                         stop=(kt == KT - 1),
                                         skip_group_check=True)
                        nc.tensor.matmul(sums[qc_][:, ds(h_ * 4 + i, 1)], stn,
                                         vn[:, kt, D:D + 1],
                                         start=(kt == 0 and i == 0 and h_ == 0),
                                         stop=(kt == KT - 1),
                                         skip_group_check=True)
            pending_pv.clear()

        def emit_quanta(qc, j, mid_fills=(2,)):
            new_pv = []
            for h in range(4):
                p, i = h // 2, h % 2
                psc = ps_sc.tile([128, 1024], F32, tag="sc", name=f"psc{qc}_{j}_{h}")
                for t in range(2):
                    kt = 2 * j + t
                    nc.tensor.matmul(psc[:, ds(t * 512, 512)],
                                     kbT[ds(i * 64, 64), ts(kt, 128)],
                                     qbT[ds(i * 64, 64), p, ds(qc * 512, 512)],
                                     start=True, stop=True)
                pt = ptp.tile([128, 1024], BF16, tag="pt", name=f"pt{qc}_{j}_{h}")
                nc.scalar.activation(out=pt, in_=psc, func=EXPF, scale=SCALE)
                new_pv.append((qc, j, h, pt))
                if h == 1:
                    flush_pv()
                if h in mid_fills:
                    fill(1)
            pending_pv.extend(new_pv)

        def alloc_state(qc):
            state[qc] = [
                ps_pv.tile([128, 2, 4, D], F32, tag="pv", name=f"pv{qc}_{hp}")
                for hp in range(2)
            ]
            sums[qc] = ps_su.tile([128, 16], F32, tag="su", name=f"su{qc}")

        def emit_norm(qc, use_act=False):
            COPYF = mybir.ActivationFunctionType.Copy
            rr = rrp.tile([128, 16], F32, tag="rr", name=f"rr{qc}")
            nc.vector.reciprocal(out=rr, in_=sums[qc])
            ao = aop.tile([128, 4, 4, D], F32R, tag="ao", name=f"ao{qc}")
            for hp in range(2):
                for hh in range(2):
                    h = 2 * hp + hh
                    for i in range(4):
                        if use_act and (i % 2 == 1):
                            nc.scalar.activation(out=ao[:, i, h, :],
                                                 in_=state[qc][hp][:, hh, i, :],
                                                 func=COPYF,
                                                 scale=rr[:, ds(h * 4 + i, 1)])
                        else:
                            nc.vector.tensor_scalar_mul(ao[:, i, h, :],
                                                        state[qc][hp][:, hh, i, :],
                                                        rr[:, ds(h * 4 + i, 1)])
                pat = ps_pj.tile([128, 512], F32R, tag="pj", name=f"pat{qc}_{hp}")
                for i in range(4):
                    nc.tensor.transpose(pat[:, ds(i * 128, 128)],
                                        ao[:, i, ds(2 * hp, 2), :], identr)
                if use_act and hp == 1:
                    nc.scalar.activation(out=aoutT[:, hp, ds(qc * 512, 512)],
                                         in_=pat, func=COPYF, scale=1.0)
                else:
                    nc.vector.tensor_copy(aoutT[:, hp, ds(qc * 512, 512)], pat)

        # ---------------- schedule ----------------
        xs_tiles = {}
        vtmp_tiles = {}

        def dma_x(sg):
            if sg in xs_tiles:
                return
            xs = xpool.tile([128, 4, DIM], F32, tag="xs", name=f"xs{sg}")
            for hlf in range(2):
                src_ap = x[ds(sg * 512 + hlf * 256, 256), :].rearrange(
                    "(i p) m -> p i m", p=128)
                nc.sync.dma_start(out=xs[:, ds(hlf * 2, 2), :], in_=src_ap)
            xb = xbp.tile([128, 4, DIM], BF16, tag="xb", name=f"xb{sg}")
            COPYF = mybir.ActivationFunctionType.Copy
            for i in range(4):
                for hlf in range(2):
                    dst = xb[:, i, ds(hlf * 512, 512)]
                    srca = xs[:, i, ds(hlf * 512, 512)]
                    if sg < 2 and hlf == 0:
                        nc.scalar.activation(out=dst, in_=srca, func=COPYF, scale=1.0)
                    elif sg >= 2 and hlf == 1:
                        nc.vector.tensor_copy(dst, srca)
                    else:
                        nc.gpsimd.tensor_copy(dst, srca)
            xs_tiles[sg] = xb

        def sgroup_units(sg, eager):
            dma_x(sg)
            units = [unit_ptrq(sg), unit_ptrp(sg, 2), unit_ptrp(sg, 3),
                     unit_pkv(sg), unit_ptv(sg)]
            if eager:
                for u in units:
                    u()
            else:
                fillq.extend(units)

        # prologue: x chunk 0 first on the DMA engines, then weights
        dma_x(0)
        nc.sync.dma_start(out=wkv_sb, in_=wkv.rearrange("(t p) m -> p t m", p=128))
        nc.sync.dma_start(out=wq_sb, in_=wq.rearrange("(t p) m -> p t m", p=128))
        sgroup_units(0, eager=True)
        unit_qt(0, 0, 0)()
        unit_qt(0, 0, 1)()
        unit_qt(0, 1, 0)()
        unit_qt(0, 1, 1)()
        nc.sync.dma_start(out=wo_sb, in_=wo.rearrange("(t p) m -> p t m", p=128).bitcast(F32R))
        alloc_state(0)
        sgroup_units(1, eager=False)
        for j in range(KT // 2):
            if j == 0:
                dma_x(2)
            if j == 1:
                sgroup_units(2, eager=False)
            if j == 2:
                dma_x(3)
            if j == 3:
                sgroup_units(3, eager=False)
            if j >= 4 and j <= 7:
                fillq.append(unit_qt(1, (j - 4) // 2, (j - 4) % 2))
            emit_quanta(0, j, mid_fills=(0, 1, 2, 3))
            fill(1)
        for qc in range(1, QC):
            emit_quanta(qc, 0)       # flushes (qc-1, 7) into state[qc-1]
            emit_norm(qc - 1)
            alloc_state(qc)
            for j in range(1, KT // 2):
                fillq.append(unit_po(qc - 1, j - 1))
                if j >= 4 and qc + 1 < QC:
                    fillq.append(unit_qt(qc + 1, (j - 4) // 2, (j - 4) % 2))
                if j == 7:
                    fillq.append(unit_po(qc - 1, 7))
                emit_quanta(qc, j)
                fill(1)
        flush_pv()
        emit_norm(QC - 1, use_act=True)
        fill(len(fillq))
        for od in range(8):
            pool = ps_pj if od % 3 == 2 else ps_sc
            unit_po(QC - 1, od, pool, use_act=True)()

    nc.compile()
    return nc


_CACHE = {}


def _get_nc(NSEQ):
    if NSEQ not in _CACHE:
        _CACHE[NSEQ] = build_nc(NSEQ)
    return _CACHE[NSEQ]


def kernel(x, Wq, Wk, Wv, Wo, bo):
    """Full-input entry point: shard over 8 cores, run, gather."""
    x, Wq, Wk, Wv, Wo, bo = (np.asarray(a, np.float32) for a in (x, Wq, Wk, Wv, Wo, bo))
    B, N, C = x.shape
    nc = _get_nc(N)
    in_maps = []
    for c in range(8):
        b, g = c // 4, c % 4
        in_maps.append({
            "x": np.ascontiguousarray(x[b]),
            "wq": np.ascontiguousarray(Wq[:, g * 256:(g + 1) * 256]).astype(BF16_NP),
            "wkv": np.ascontiguousarray(np.concatenate(
                [Wk[:, g * D:(g + 1) * D], Wv[:, g * D:(g + 1) * D]],
                axis=1)).astype(BF16_NP),
            "wo": np.ascontiguousarray(Wo[g * 256:(g + 1) * 256, :]),
        })
    res = bass_utils.run_bass_kernel_spmd(nc, in_maps, core_ids=list(range(8)))
    outs = [res.results[c]["out"] for c in range(8)]
    full = np.empty((B, N, C), np.float32)
    for b in range(B):
        acc = outs[4 * b].astype(np.float32)
        for g in range(1, 4):
            acc = acc + outs[4 * b + g]
        full[b] = acc.T + bo[None, :]
    return full


# revision 33
# speedup vs baseline: 1.0055x; 1.0054x over previous
"""GQA attention kernel for Trainium2 (Bass/Tile), 8-core SPMD.

Problem: B=2, N=2048, DIM=1024, 16 query heads / 4 KV heads, head_dim=64, fp32.
Sharding: core c = (batch b=c//4, kv-group g=c%4). Each core computes its
group's 4 query heads + 1 shared KV head over the full sequence, and a partial
output projection (its 256 rows of Wo). Host sums the 4 group partials per
batch and adds the bias.

Layout per core:
  xT    [128, 8, N] f32r : x^T (PE transposes with an f32r identity)
  qt    [128, 2, N] f32r : Q^T head pairs (head 2p on partitions 0-63, 2p+1 on
                           64-127)
  kkT   [128, N]    f32r : K^T duplicated across partition halves (DMA dup)
  vn    [128, 16, 65] bf16: V in normal layout (keys on partitions) + ones col
  aoutT [128, 2, N] f32r : normalized attention out^T for the out-projection

Scores are computed transposed (S^T [128 keys, 512 queries]); exp on Act; P@V
uses P^T tiles as the *stationary* operand and V as the moving operand,
producing [queries, 64] in PSUM at 64 rows/matmul instead of 128; sum-of-exp
rides on 1-row ones-matmuls into a dedicated PSUM bank.

PSUM budget (8 banks): scores 2x[128,1024] double-buffered (4) + P@V
accumulators 2x[128,512] (2, two heads per bank) + transpose staging (1) +
sum-of-exp (1). Projection matmuls share the score pool, interleaved
fine-grained between score tiles so no engine convoys behind one pool.
"""

import sys

if "/opt/trn_rl_repo" not in sys.path:
    sys.path.insert(0, "/opt/trn_rl_repo")

from collections import deque
from contextlib import ExitStack

import ml_dtypes
import numpy as np

BF16_NP = ml_dtypes.bfloat16

import concourse.bass as bass
import concourse.mybir as mybir
import concourse.tile as tile
from concourse import bacc, bass_utils
from concourse.bass import ds, ts
from concourse.masks import make_identity

F32 = mybir.dt.float32
F32R = mybir.dt.float32r
BF16 = mybir.dt.bfloat16
FP8 = mybir.dt.float8e4
DROW = mybir.MatmulPerfMode.DoubleRow
EXPF = mybir.ActivationFunctionType.Exp

DIM = 1024
D = 64  # head dim
SCALE = D ** -0.5


def build_nc(NSEQ=2048):
    KT = NSEQ // 128   # key tiles
    QC = NSEQ // 512   # query chunks of 512
    DKT = DIM // 128   # contraction tiles for projections

    nc = bacc.Bacc("TRN2", target_bir_lowering=False, debug=False)
    x = nc.dram_tensor("x", [NSEQ, DIM], F32, kind="ExternalInput").ap()
    wq = nc.dram_tensor("wq", [DIM, 256], BF16, kind="ExternalInput").ap()
    wkv = nc.dram_tensor("wkv", [DIM, 128], BF16, kind="ExternalInput").ap()
    wo = nc.dram_tensor("wo", [256, DIM], F32, kind="ExternalInput").ap()
    out = nc.dram_tensor("out", [DIM, NSEQ], F32, kind="ExternalOutput").ap()

    with tile.TileContext(nc) as tc, ExitStack() as ctx:
        sb = ctx.enter_context(tc.tile_pool(name="sb", bufs=1))

        wq_sb = sb.tile([128, DKT, 256], BF16)
        wkv_sb = sb.tile([128, DKT, 128], BF16)
        wo_sb = sb.tile([128, 2, DIM], F32R)
        ident = sb.tile([128, 128], F32)
        identr = sb.tile([128, 128], F32R)
        identb = sb.tile([128, 128], BF16)
        warm_in = sb.tile([128, 1], F32)
        warm = sb.tile([128, 1], F32)

        make_identity(nc, ident)
        nc.vector.tensor_copy(identr, ident)
        nc.vector.tensor_copy(identb, ident)
        nc.vector.memset(warm_in, 1.0)
        # preload the exp table set off the critical path
        nc.scalar.activation(out=warm, in_=warm_in, func=EXPF, scale=1.0)

        xT = sb.tile([128, DKT, NSEQ], BF16)
        qbT = sb.tile([128, 2, NSEQ], BF16)
        kbT = sb.tile([128, NSEQ], BF16)
        vn = sb.tile([128, KT, D + 1], BF16)
        aoutT = sb.tile([128, 2, NSEQ], F32R)
        nc.vector.memset(vn, 1.0)

        xpool = ctx.enter_context(tc.tile_pool(name="xp", bufs=3))
        xbp = ctx.enter_context(tc.tile_pool(name="xb", bufs=3))
        vtp = ctx.enter_context(tc.tile_pool(name="vtp", bufs=2))
        ptp = ctx.enter_context(tc.tile_pool(name="ptp", bufs=12))
        rrp = ctx.enter_context(tc.tile_pool(name="rrp", bufs=2))
        aop = ctx.enter_context(tc.tile_pool(name="aop", bufs=2))
        outp = ctx.enter_context(tc.tile_pool(name="outp", bufs=4))
        # PSUM: ps_sc 2x[128,1024] (banks 0-3), ps_pv 2x[128,512] (4-5),
        # ps_pj 1x[128,512] (6), ps_su 1x[128,16] (7)
        ps_sc = ctx.enter_context(tc.tile_pool(name="ps_sc", bufs=2, space="PSUM"))
        ps_pv = ctx.enter_context(tc.tile_pool(name="ps_pv", bufs=2, space="PSUM"))
        ps_pj = ctx.enter_context(tc.tile_pool(name="ps_pj", bufs=1, space="PSUM"))
        ps_su = ctx.enter_context(tc.tile_pool(name="ps_su", bufs=1, space="PSUM"))

        state = {}     # qc -> [hp0_tile, hp1_tile] each [128, 2, 4, 64] view
        sums = {}      # qc -> [128, 16] psum tile (cols h*4+i)
        pending_pv = []
        fillq = deque()

        def fill(n=1):
            for _ in range(n):
                if not fillq:
                    return
                fillq.popleft()()

        # ---------------- work units ----------------
        def unit_ptrq(sg):
            """Transpose x DIM-tiles 0-3 of chunk sg into one 2-bank bf16 psum."""
            def run():
                ptr = ps_sc.tile([128, 2048], BF16, tag="sc", name=f"ptrq{sg}")
                for d in range(4):
                    for i in range(4):
                        nc.tensor.transpose(ptr[:, ds(d * 512 + i * 128, 128)],
                                            xs_tiles[sg][:, i, ts(d, 128)], identb)
                nc.vector.tensor_copy(xT[:, ds(0, 4), ds(sg * 512, 512)], ptr)
            return run

        def unit_ptrp(sg, dp):
            """Transpose x DIM-tiles 2dp..2dp+1 into one 1-bank bf16 psum."""
            def run():
                ptr = ps_pj.tile([128, 1024], BF16, tag="pj", name=f"ptrp{sg}_{dp}")
                for k in range(2):
                    d = 2 * dp + k
                    for i in range(4):
                        nc.tensor.transpose(ptr[:, ds(k * 512 + i * 128, 128)],
                                            xs_tiles[sg][:, i, ts(d, 128)], identb)
                nc.vector.tensor_copy(xT[:, ds(2 * dp, 2), ds(sg * 512, 512)], ptr)
            return run

        def unit_pkv(sg):
            def run():
                pkv = ps_pj.tile([128, 512], F32, tag="pj", name=f"pkv{sg}")
                for d in range(DKT):
                    nc.tensor.matmul(pkv[:, 0:512], wkv_sb[:, d, :],
                                     xT[:, d, ds(sg * 512, 512)],
                                     start=(d == 0), stop=(d == DKT - 1))
                nc.vector.tensor_copy(kbT[ds(0, 64), ds(sg * 512, 512)],
                                      pkv[ds(0, 64), 0:512])
                nc.sync.dma_start(out=kbT[ds(64, 64), ds(sg * 512, 512)],
                                  in_=kbT[ds(0, 64), ds(sg * 512, 512)])
                vtmp = vtp.tile([64, 512], BF16, tag="vt", name=f"vt{sg}")
                nc.vector.tensor_copy(vtmp, pkv[ds(64, 64), 0:512])
                vtmp_tiles[sg] = vtmp
            return run

        def unit_ptv(sg):
            def run():
                ptv = ps_pj.tile([128, 1024], BF16, tag="pj", name=f"ptv{sg}")
                for i in range(4):
                    nc.tensor.transpose(ptv[:, ds(i * D, D)], vtmp_tiles[sg][:, ts(i, 128)],
                                        identb[0:64, 0:64])
                nc.vector.tensor_copy(vn[:, ds(sg * 4, 4), 0:D], ptv[:, 0:4 * D])
            return run

        def unit_qt(qc, p, hlf=None):
            def run():
                if hlf is None:
                    qw, off = 512, 0
                else:
                    qw, off = 256, hlf * 256
                pq = ps_sc.tile([128, 1024], F32, tag="sc", name=f"pq{qc}_{p}_{off}")
                for d in range(DKT):
                    nc.tensor.matmul(pq[:, 0:qw], wq_sb[:, d, ts(p, 128)],
                                     xT[:, d, ds(qc * 512 + off, qw)],
                                     start=(d == 0), stop=(d == DKT - 1))
                nc.vector.tensor_copy(qbT[:, p, ds(qc * 512 + off, qw)], pq[:, 0:qw])
            return run

        def unit_po(qc, od, pool=None, use_act=False):
            def run():
                p_ = pool if pool is not None else ps_sc
                tag = "pj" if p_ is ps_pj else "sc"
                shape = [128, 512] if p_ is ps_pj else [128, 1024]
                po = p_.tile(shape, F32, tag=tag, name=f"po{qc}_{od}")
                nc.tensor.matmul(po[:, 0:512], wo_sb[:, 0, ts(od, 128)],
                                 aoutT[:, 0, ds(qc * 512, 512)], start=True, stop=False)
                nc.tensor.matmul(po[:, 0:512], wo_sb[:, 1, ts(od, 128)],
                                 aoutT[:, 1, ds(qc * 512, 512)], start=False, stop=True)
                ot = outp.tile([128, 512], F32, tag="ot", name=f"ot{qc}_{od}")
                if use_act and od % 2 == 1:
                    nc.scalar.activation(out=ot, in_=po[:, 0:512],
                                         func=mybir.ActivationFunctionType.Copy,
                                         scale=1.0)
                else:
                    nc.vector.tensor_copy(ot, po[:, 0:512])
                nc.sync.dma_start(out=out[ts(od, 128), ds(qc * 512, 512)], in_=ot)
            return run

        # ---------------- attention ----------------
        def flush_pv():
            for (qc_, j_, h_, pt_) in pending_pv:
                hp, hh = h_ // 2, h_ % 2
                for t in range(2):
                    kt = 2 * j_ + t
                    for i in range(4):
                        stn = pt_[:, ds(t * 512 + i * 128, 128)]
                        # start=True zeroes the whole 2KB PSUM bank: only the
                        # first series touching each bank may set it.
                        nc.tensor.matmul(state[qc_][hp][:, hh, i, :], stn,
                                         vn[:, kt, 0:D],
                                         start=(kt == 0 and i == 0 and hh == 0),
                                         stop=(kt == KT - 1),
                                         skip_group_check=True)
                        nc.tensor.matmul(sums[qc_][:, ds(h_ * 4 + i, 1)], stn,
                                         vn[:, kt, D:D + 1],
                                         start=(kt == 0 and i == 0 and h_ == 0),
                                         stop=(kt == KT - 1),
                                         skip_group_check=True)
            pending_pv.clear()

        def emit_quanta(qc, j, mid_fills=(2,)):
            new_pv = []
            for h in range(4):
                p, i = h // 2, h % 2
                psc = ps_sc.tile([128, 1024], F32, tag="sc", name=f"psc{qc}_{j}_{h}")
                for t in range(2):
                    kt = 2 * j + t
                    nc.tensor.matmul(psc[:, ds(t * 512, 512)],
                                     kbT[ds(i * 64, 64), ts(kt, 128)],
                                     qbT[ds(i * 64, 64), p, ds(qc * 512, 512)],
                                     start=True, stop=True)
                pt = ptp.tile([128, 1024], BF16, tag="pt", name=f"pt{qc}_{j}_{h}")
                nc.scalar.activation(out=pt, in_=psc, func=EXPF, scale=SCALE)
                new_pv.append((qc, j, h, pt))
                if h == 1:
                    flush_pv()
                if h in mid_fills:
                    fill(1)
            pending_pv.extend(new_pv)

        def alloc_state(qc):
            state[qc] = [
                ps_pv.tile([128, 2, 4, D], F32, tag="pv", name=f"pv{qc}_{hp}")
                for hp in range(2)
            ]
            sums[qc] = ps_su.tile([128, 16], F32, tag="su", name=f"su{qc}")

        def emit_norm(qc, use_act=False):
            COPYF = mybir.ActivationFunctionType.Copy
            rr = rrp.tile([128, 16], F32, tag="rr", name=f"rr{qc}")
            nc.vector.reciprocal(out=rr, in_=sums[qc])
            ao = aop.tile([128, 4, 4, D], F32R, tag="ao", name=f"ao{qc}")
            for hp in range(2):
                for hh in range(2):
                    h = 2 * hp + hh
                    for i in range(4):
                        if use_act and (i % 2 == 1):
                            nc.scalar.activation(out=ao[:, i, h, :],
                                                 in_=state[qc][hp][:, hh, i, :],
                                                 func=COPYF,
                                                 scale=rr[:, ds(h * 4 + i, 1)])
                        else:
                            nc.vector.tensor_scalar_mul(ao[:, i, h, :],
                                                        state[qc][hp][:, hh, i, :],
                                                        rr[:, ds(h * 4 + i, 1)])
                pat = ps_pj.tile([128, 512], F32R, tag="pj", name=f"pat{qc}_{hp}")
                for i in range(4):
                    nc.tensor.transpose(pat[:, ds(i * 128, 128)],
                                        ao[:, i, ds(2 * hp, 2), :], identr)
                if use_act and hp == 1:
                    nc.scalar.activation(out=aoutT[:, hp, ds(qc * 512, 512)],
                                         in_=pat, func=COPYF, scale=1.0)
                else:
                    nc.vector.tensor_copy(aoutT[:, hp, ds(qc * 512, 512)], pat)

        # ---------------- schedule ----------------
        xs_tiles = {}
        vtmp_tiles = {}

        def dma_x(sg):
            if sg in xs_tiles:
                return
            xs = xpool.tile([128, 4, DIM], F32, tag="xs", name=f"xs{sg}")
            for hlf in range(2):
                src_ap = x[ds(sg * 512 + hlf * 256, 256), :].rearrange(
                    "(i p) m -> p i m", p=128)
                nc.sync.dma_start(out=xs[:, ds(hlf * 2, 2), :], in_=src_ap)
            xb = xbp.tile([128, 4, DIM], BF16, tag="xb", name=f"xb{sg}")
            COPYF = mybir.ActivationFunctionType.Copy
            for i in range(4):
                for hlf in range(2):
                    dst = xb[:, i, ds(hlf * 512, 512)]
                    srca = xs[:, i, ds(hlf * 512, 512)]
                    if sg < 2 and hlf == 0:
                        nc.scalar.activation(out=dst, in_=srca, func=COPYF, scale=1.0)
                    elif sg >= 2 and hlf == 1:
                        nc.vector.tensor_copy(dst, srca)
                    else:
                        nc.gpsimd.tensor_copy(dst, srca)
            xs_tiles[sg] = xb

        def sgroup_units(sg, eager):
            dma_x(sg)
            units = [unit_ptrq(sg), unit_ptrp(sg, 2), unit_ptrp(sg, 3),
                     unit_pkv(sg), unit_ptv(sg)]
            if eager:
                for u in units:
                    u()
            else:
                fillq.extend(units)

        # prologue: x chunk 0 first on the DMA engines, then weights
        dma_x(0)
        nc.sync.dma_start(out=wkv_sb, in_=wkv.rearrange("(t p) m -> p t m", p=128))
        nc.sync.dma_start(out=wq_sb, in_=wq.rearrange("(t p) m -> p t m", p=128))
        sgroup_units(0, eager=True)
        unit_qt(0, 0, 0)()
        unit_qt(0, 0, 1)()
        unit_qt(0, 1, 0)()
        unit_qt(0, 1, 1)()
        nc.sync.dma_start(out=wo_sb, in_=wo.rearrange("(t p) m -> p t m", p=128).bitcast(F32R))
        alloc_state(0)
        sgroup_units(1, eager=False)
        for j in range(KT // 2):
            if j == 0:
                dma_x(2)
            if j == 1:
                sgroup_units(2, eager=False)
            if j == 2:
                dma_x(3)
            if j == 3:
                sgroup_units(3, eager=False)
            if j >= 4 and j <= 7:
                fillq.append(unit_qt(1, (j - 4) // 2, (j - 4) % 2))
            emit_quanta(0, j, mid_fills=(0, 2))
            fill(1)
        for qc in range(1, QC):
            emit_quanta(qc, 0)       # flushes (qc-1, 7) into state[qc-1]
            emit_norm(qc - 1)
            alloc_state(qc)
            for j in range(1, KT // 2):
                fillq.append(unit_po(qc - 1, j - 1))
                if j >= 4 and qc + 1 < QC:
                    fillq.append(unit_qt(qc + 1, (j - 4) // 2, (j - 4) % 2))
                if j == 7:
                    fillq.append(unit_po(qc - 1, 7))
                emit_quanta(qc, j)
                fill(1)
        flush_pv()
        emit_norm(QC - 1, use_act=True)
        fill(len(fillq))
        for od in range(8):
            pool = ps_pj if od % 3 == 2 else ps_sc
            unit_po(QC - 1, od, pool, use_act=True)()

    nc.compile()
    return nc


_CACHE = {}


def _get_nc(NSEQ):
    if NSEQ not in _CACHE:
        _CACHE[NSEQ] = build_nc(NSEQ)
    return _CACHE[NSEQ]


def kernel(x, Wq, Wk, Wv, Wo, bo):
    """Full-input entry point: shard over 8 cores, run, gather."""
    x, Wq, Wk, Wv, Wo, bo = (np.asarray(a, np.float32) for a in (x, Wq, Wk, Wv, Wo, bo))
    B, N, C = x.shape
    nc = _get_nc(N)
    in_maps = []
    for c in range(8):
        b, g = c // 4, c % 4
        in_maps.append({
            "x": np.ascontiguousarray(x[b]),
            "wq": np.ascontiguousarray(Wq[:, g * 256:(g + 1) * 256]).astype(BF16_NP),
            "wkv": np.ascontiguousarray(np.concatenate(
                [Wk[:, g * D:(g + 1) * D], Wv[:, g * D:(g + 1) * D]],
                axis=1)).astype(BF16_NP),
            "wo": np.ascontiguousarray(Wo[g * 256:(g + 1) * 256, :]),
        })
    res = bass_utils.run_bass_kernel_spmd(nc, in_maps, core_ids=list(range(8)))
    outs = [res.results[c]["out"] for c in range(8)]
    full = np.empty((B, N, C), np.float32)
    for b in range(B):
        acc = outs[4 * b].astype(np.float32)
        for g in range(1, 4):
            acc = acc + outs[4 * b + g]
        full[b] = acc.T + bo[None, :]
    return full


# revision 34
# speedup vs baseline: 1.0097x; 1.0042x over previous
"""GQA attention kernel for Trainium2 (Bass/Tile), 8-core SPMD.

Problem: B=2, N=2048, DIM=1024, 16 query heads / 4 KV heads, head_dim=64, fp32.
Sharding: core c = (batch b=c//4, kv-group g=c%4). Each core computes its
group's 4 query heads + 1 shared KV head over the full sequence, and a partial
output projection (its 256 rows of Wo). Host sums the 4 group partials per
batch and adds the bias.

Layout per core:
  xT    [128, 8, N] f32r : x^T (PE transposes with an f32r identity)
  qt    [128, 2, N] f32r : Q^T head pairs (head 2p on partitions 0-63, 2p+1 on
                           64-127)
  kkT   [128, N]    f32r : K^T duplicated across partition halves (DMA dup)
  vn    [128, 16, 65] bf16: V in normal layout (keys on partitions) + ones col
  aoutT [128, 2, N] f32r : normalized attention out^T for the out-projection

Scores are computed transposed (S^T [128 keys, 512 queries]); exp on Act; P@V
uses P^T tiles as the *stationary* operand and V as the moving operand,
producing [queries, 64] in PSUM at 64 rows/matmul instead of 128; sum-of-exp
rides on 1-row ones-matmuls into a dedicated PSUM bank.

PSUM budget (8 banks): scores 2x[128,1024] double-buffered (4) + P@V
accumulators 2x[128,512] (2, two heads per bank) + transpose staging (1) +
sum-of-exp (1). Projection matmuls share the score pool, interleaved
fine-grained between score tiles so no engine convoys behind one pool.
"""

import sys

if "/opt/trn_rl_repo" not in sys.path:
    sys.path.insert(0, "/opt/trn_rl_repo")

from collections import deque
from contextlib import ExitStack

import ml_dtypes
import numpy as np

BF16_NP = ml_dtypes.bfloat16

import concourse.bass as bass
import concourse.mybir as mybir
import concourse.tile as tile
from concourse import bacc, bass_utils
from concourse.bass import ds, ts
from concourse.masks import make_identity

F32 = mybir.dt.float32
F32R = mybir.dt.float32r
BF16 = mybir.dt.bfloat16
FP8 = mybir.dt.float8e4
DROW = mybir.MatmulPerfMode.DoubleRow
EXPF = mybir.ActivationFunctionType.Exp

DIM = 1024
D = 64  # head dim
SCALE = D ** -0.5


def build_nc(NSEQ=2048):
    KT = NSEQ // 128   # key tiles
    QC = NSEQ // 512   # query chunks of 512
    DKT = DIM // 128   # contraction tiles for projections

    nc = bacc.Bacc("TRN2", target_bir_lowering=False, debug=False)
    x = nc.dram_tensor("x", [NSEQ, DIM], F32, kind="ExternalInput").ap()
    wq = nc.dram_tensor("wq", [DIM, 256], BF16, kind="ExternalInput").ap()
    wkv = nc.dram_tensor("wkv", [DIM, 128], BF16, kind="ExternalInput").ap()
    wo = nc.dram_tensor("wo", [256, DIM], F32, kind="ExternalInput").ap()
    out = nc.dram_tensor("out", [DIM, NSEQ], F32, kind="ExternalOutput").ap()

    with tile.TileContext(nc) as tc, ExitStack() as ctx:
        sb = ctx.enter_context(tc.tile_pool(name="sb", bufs=1))

        wq_sb = sb.tile([128, DKT, 256], BF16)
        wkv_sb = sb.tile([128, DKT, 128], BF16)
        wo_sb = sb.tile([128, 2, DIM], F32R)
        ident = sb.tile([128, 128], F32)
        identr = sb.tile([128, 128], F32R)
        identb = sb.tile([128, 128], BF16)
        warm_in = sb.tile([128, 1], F32)
        warm = sb.tile([128, 1], F32)

        make_identity(nc, ident)
        nc.vector.tensor_copy(identr, ident)
        nc.vector.tensor_copy(identb, ident)
        nc.vector.memset(warm_in, 1.0)
        # preload the exp table set off the critical path
        nc.scalar.activation(out=warm, in_=warm_in, func=EXPF, scale=1.0)

        xT = sb.tile([128, DKT, NSEQ], BF16)
        qbT = sb.tile([128, 2, NSEQ], BF16)
        kbT = sb.tile([128, NSEQ], BF16)
        vn = sb.tile([128, KT, D + 1], BF16)
        aoutT = sb.tile([128, 2, NSEQ], F32R)
        nc.vector.memset(vn, 1.0)

        xpool = ctx.enter_context(tc.tile_pool(name="xp", bufs=3))
        xbp = ctx.enter_context(tc.tile_pool(name="xb", bufs=3))
        vtp = ctx.enter_context(tc.tile_pool(name="vtp", bufs=2))
        ptp = ctx.enter_context(tc.tile_pool(name="ptp", bufs=12))
        rrp = ctx.enter_context(tc.tile_pool(name="rrp", bufs=2))
        aop = ctx.enter_context(tc.tile_pool(name="aop", bufs=2))
        outp = ctx.enter_context(tc.tile_pool(name="outp", bufs=4))
        # PSUM: ps_sc 2x[128,1024] (banks 0-3), ps_pv 2x[128,512] (4-5),
        # ps_pj 1x[128,512] (6), ps_su 1x[128,16] (7)
        ps_sc = ctx.enter_context(tc.tile_pool(name="ps_sc", bufs=2, space="PSUM"))
        ps_pv = ctx.enter_context(tc.tile_pool(name="ps_pv", bufs=2, space="PSUM"))
        ps_pj = ctx.enter_context(tc.tile_pool(name="ps_pj", bufs=1, space="PSUM"))
        ps_su = ctx.enter_context(tc.tile_pool(name="ps_su", bufs=1, space="PSUM"))

        state = {}     # qc -> [hp0_tile, hp1_tile] each [128, 2, 4, 64] view
        sums = {}      # qc -> [128, 16] psum tile (cols h*4+i)
        pending_pv = []
        fillq = deque()

        def fill(n=1):
            for _ in range(n):
                if not fillq:
                    return
                fillq.popleft()()

        # ---------------- work units ----------------
        def unit_ptrq(sg):
            """Transpose x DIM-tiles 0-3 of chunk sg into one 2-bank bf16 psum."""
            def run():
                ptr = ps_sc.tile([128, 2048], BF16, tag="sc", name=f"ptrq{sg}")
                for d in range(4):
                    for i in range(4):
                        nc.tensor.transpose(ptr[:, ds(d * 512 + i * 128, 128)],
                                            xs_tiles[sg][:, i, ts(d, 128)], identb)
                nc.vector.tensor_copy(xT[:, ds(0, 4), ds(sg * 512, 512)], ptr)
            return run

        def unit_ptrp(sg, dp):
            """Transpose x DIM-tiles 2dp..2dp+1 into one 1-bank bf16 psum."""
            def run():
                ptr = ps_pj.tile([128, 1024], BF16, tag="pj", name=f"ptrp{sg}_{dp}")
                for k in range(2):
                    d = 2 * dp + k
                    for i in range(4):
                        nc.tensor.transpose(ptr[:, ds(k * 512 + i * 128, 128)],
                                            xs_tiles[sg][:, i, ts(d, 128)], identb)
                nc.vector.tensor_copy(xT[:, ds(2 * dp, 2), ds(sg * 512, 512)], ptr)
            return run

        def unit_pkv(sg):
            def run():
                pkv = ps_pj.tile([128, 512], F32, tag="pj", name=f"pkv{sg}")
                for d in range(DKT):
                    nc.tensor.matmul(pkv[:, 0:512], wkv_sb[:, d, :],
                                     xT[:, d, ds(sg * 512, 512)],
                                     start=(d == 0), stop=(d == DKT - 1))
                nc.vector.tensor_copy(kbT[ds(0, 64), ds(sg * 512, 512)],
                                      pkv[ds(0, 64), 0:512])
                nc.sync.dma_start(out=kbT[ds(64, 64), ds(sg * 512, 512)],
                                  in_=kbT[ds(0, 64), ds(sg * 512, 512)])
                vtmp = vtp.tile([64, 512], BF16, tag="vt", name=f"vt{sg}")
                nc.vector.tensor_copy(vtmp, pkv[ds(64, 64), 0:512])
                vtmp_tiles[sg] = vtmp
            return run

        def unit_ptv(sg):
            def run():
                ptv = ps_pj.tile([128, 1024], BF16, tag="pj", name=f"ptv{sg}")
                for i in range(4):
                    nc.tensor.transpose(ptv[:, ds(i * D, D)], vtmp_tiles[sg][:, ts(i, 128)],
                                        identb[0:64, 0:64])
                nc.vector.tensor_copy(vn[:, ds(sg * 4, 4), 0:D], ptv[:, 0:4 * D])
            return run

        def unit_qt(qc, p, hlf=None):
            def run():
                if hlf is None:
                    qw, off = 512, 0
                else:
                    qw, off = 256, hlf * 256
                pq = ps_sc.tile([128, 1024], F32, tag="sc", name=f"pq{qc}_{p}_{off}")
                for d in range(DKT):
                    nc.tensor.matmul(pq[:, 0:qw], wq_sb[:, d, ts(p, 128)],
                                     xT[:, d, ds(qc * 512 + off, qw)],
                                     start=(d == 0), stop=(d == DKT - 1))
                nc.vector.tensor_copy(qbT[:, p, ds(qc * 512 + off, qw)], pq[:, 0:qw])
            return run

        def unit_po(qc, od, pool=None, use_act=False):
            def run():
                p_ = pool if pool is not None else ps_sc
                tag = "pj" if p_ is ps_pj else "sc"
                shape = [128, 512] if p_ is ps_pj else [128, 1024]
                po = p_.tile(shape, F32, tag=tag, name=f"po{qc}_{od}")
                nc.tensor.matmul(po[:, 0:512], wo_sb[:, 0, ts(od, 128)],
                                 aoutT[:, 0, ds(qc * 512, 512)], start=True, stop=False)
                nc.tensor.matmul(po[:, 0:512], wo_sb[:, 1, ts(od, 128)],
                                 aoutT[:, 1, ds(qc * 512, 512)], start=False, stop=True)
                ot = outp.tile([128, 512], F32, tag="ot", name=f"ot{qc}_{od}")
                if use_act and od % 2 == 1:
                    nc.scalar.activation(out=ot, in_=po[:, 0:512],
                                         func=mybir.ActivationFunctionType.Copy,
                                         scale=1.0)
                else:
                    nc.vector.tensor_copy(ot, po[:, 0:512])
                nc.sync.dma_start(out=out[ts(od, 128), ds(qc * 512, 512)], in_=ot)
            return run

        # ---------------- attention ----------------
        def flush_pv():
            for (qc_, j_, h_, pt_) in pending_pv:
                hp, hh = h_ // 2, h_ % 2
                for t in range(2):
                    kt = 2 * j_ + t
                    for i in range(4):
                        stn = pt_[:, ds(t * 512 + i * 128, 128)]
                        # start=True zeroes the whole 2KB PSUM bank: only the
                        # first series touching each bank may set it.
                        nc.tensor.matmul(state[qc_][hp][:, hh, i, :], stn,
                                         vn[:, kt, 0:D],
                                         start=(kt == 0 and i == 0 and hh == 0),
                                         stop=(kt == KT - 1),
                                         skip_group_check=True)
                        nc.tensor.matmul(sums[qc_][:, ds(h_ * 4 + i, 1)], stn,
                                         vn[:, kt, D:D + 1],
                                         start=(kt == 0 and i == 0 and h_ == 0),
                                         stop=(kt == KT - 1),
                                         skip_group_check=True)
            pending_pv.clear()

        def emit_quanta(qc, j, mid_fills=(2,)):
            new_pv = []
            for h in range(4):
                p, i = h // 2, h % 2
                psc = ps_sc.tile([128, 1024], F32, tag="sc", name=f"psc{qc}_{j}_{h}")
                for t in range(2):
                    kt = 2 * j + t
                    nc.tensor.matmul(psc[:, ds(t * 512, 512)],
                                     kbT[ds(i * 64, 64), ts(kt, 128)],
                                     qbT[ds(i * 64, 64), p, ds(qc * 512, 512)],
                                     start=True, stop=True)
                pt = ptp.tile([128, 1024], BF16, tag="pt", name=f"pt{qc}_{j}_{h}")
                nc.scalar.activation(out=pt, in_=psc, func=EXPF, scale=SCALE)
                new_pv.append((qc, j, h, pt))
                if h == 1:
                    flush_pv()
                if h in mid_fills:
                    fill(1)
            pending_pv.extend(new_pv)

        def alloc_state(qc):
            state[qc] = [
                ps_pv.tile([128, 2, 4, D], F32, tag="pv", name=f"pv{qc}_{hp}")
                for hp in range(2)
            ]
            sums[qc] = ps_su.tile([128, 16], F32, tag="su", name=f"su{qc}")

        def emit_norm(qc, use_act=False):
            COPYF = mybir.ActivationFunctionType.Copy
            rr = rrp.tile([128, 16], F32, tag="rr", name=f"rr{qc}")
            nc.vector.reciprocal(out=rr, in_=sums[qc])
            ao = aop.tile([128, 4, 4, D], F32R, tag="ao", name=f"ao{qc}")
            for hp in range(2):
                for hh in range(2):
                    h = 2 * hp + hh
                    for i in range(4):
                        if use_act and (i % 2 == 1):
                            nc.scalar.activation(out=ao[:, i, h, :],
                                                 in_=state[qc][hp][:, hh, i, :],
                                                 func=COPYF,
                                                 scale=rr[:, ds(h * 4 + i, 1)])
                        else:
                            nc.vector.tensor_scalar_mul(ao[:, i, h, :],
                                                        state[qc][hp][:, hh, i, :],
                                                        rr[:, ds(h * 4 + i, 1)])
                pat = ps_pj.tile([128, 512], F32R, tag="pj", name=f"pat{qc}_{hp}")
                for i in range(4):
                    nc.tensor.transpose(pat[:, ds(i * 128, 128)],
                                        ao[:, i, ds(2 * hp, 2), :], identr)
                if use_act and hp == 1:
                    nc.scalar.activation(out=aoutT[:, hp, ds(qc * 512, 512)],
                                         in_=pat, func=COPYF, scale=1.0)
                else:
                    nc.vector.tensor_copy(aoutT[:, hp, ds(qc * 512, 512)], pat)

        # ---------------- schedule ----------------
        xs_tiles = {}
        vtmp_tiles = {}

        def dma_x(sg):
            if sg in xs_tiles:
                return
            xs = xpool.tile([128, 4, DIM], F32, tag="xs", name=f"xs{sg}")
            for hlf in range(2):
                src_ap = x[ds(sg * 512 + hlf * 256, 256), :].rearrange(
                    "(i p) m -> p i m", p=128)
                nc.sync.dma_start(out=xs[:, ds(hlf * 2, 2), :], in_=src_ap)
            xb = xbp.tile([128, 4, DIM], BF16, tag="xb", name=f"xb{sg}")
            COPYF = mybir.ActivationFunctionType.Copy
            for i in range(4):
                for hlf in range(2):
                    dst = xb[:, i, ds(hlf * 512, 512)]
                    srca = xs[:, i, ds(hlf * 512, 512)]
                    if sg < 2 and hlf == 0:
                        nc.scalar.activation(out=dst, in_=srca, func=COPYF, scale=1.0)
                    elif sg >= 2 and hlf == 1:
                        nc.vector.tensor_copy(dst, srca)
                    else:
                        nc.gpsimd.tensor_copy(dst, srca)
            xs_tiles[sg] = xb

        def sgroup_units(sg, eager):
            dma_x(sg)
            units = [unit_ptrq(sg), unit_ptrp(sg, 2), unit_ptrp(sg, 3),
                     unit_pkv(sg), unit_ptv(sg)]
            if eager:
                for u in units:
                    u()
            else:
                fillq.extend(units)

        # prologue: x chunk 0 first on the DMA engines, then weights
        dma_x(0)
        nc.sync.dma_start(out=wkv_sb, in_=wkv.rearrange("(t p) m -> p t m", p=128))
        nc.sync.dma_start(out=wq_sb, in_=wq.rearrange("(t p) m -> p t m", p=128))
        sgroup_units(0, eager=True)
        unit_qt(0, 0, 0)()
        unit_qt(0, 0, 1)()
        unit_qt(0, 1, 0)()
        unit_qt(0, 1, 1)()
        nc.sync.dma_start(out=wo_sb, in_=wo.rearrange("(t p) m -> p t m", p=128).bitcast(F32R))
        alloc_state(0)
        sgroup_units(1, eager=False)
        for j in range(KT // 2):
            if j == 0:
                dma_x(2)
            if j == 1:
                sgroup_units(2, eager=False)
            if j == 2:
                dma_x(3)
            if j == 3:
                sgroup_units(3, eager=False)
            if j >= 4 and j <= 7:
                fillq.append(unit_qt(1, (j - 4) // 2, (j - 4) % 2))
            emit_quanta(0, j, mid_fills=(1, 3))
            fill(1)
        for qc in range(1, QC):
            emit_quanta(qc, 0)       # flushes (qc-1, 7) into state[qc-1]
            emit_norm(qc - 1)
            alloc_state(qc)
            for j in range(1, KT // 2):
                fillq.append(unit_po(qc - 1, j - 1))
                if j >= 4 and qc + 1 < QC:
                    fillq.append(unit_qt(qc + 1, (j - 4) // 2, (j - 4) % 2))
                if j == 7:
                    fillq.append(unit_po(qc - 1, 7))
                emit_quanta(qc, j)
                fill(1)
        flush_pv()
        emit_norm(QC - 1, use_act=True)
        fill(len(fillq))
        for od in range(8):
            pool = ps_pj if od % 3 == 2 else ps_sc
            unit_po(QC - 1, od, pool, use_act=True)()

    nc.compile()
    return nc


_CACHE = {}


def _get_nc(NSEQ):
    if NSEQ not in _CACHE:
        _CACHE[NSEQ] = build_nc(NSEQ)
    return _CACHE[NSEQ]


def kernel(x, Wq, Wk, Wv, Wo, bo):
    """Full-input entry point: shard over 8 cores, run, gather."""
    x, Wq, Wk, Wv, Wo, bo = (np.asarray(a, np.float32) for a in (x, Wq, Wk, Wv, Wo, bo))
    B, N, C = x.shape
    nc = _get_nc(N)
    in_maps = []
    for c in range(8):
        b, g = c // 4, c % 4
        in_maps.append({
            "x": np.ascontiguousarray(x[b]),
            "wq": np.ascontiguousarray(Wq[:, g * 256:(g + 1) * 256]).astype(BF16_NP),
            "wkv": np.ascontiguousarray(np.concatenate(
                [Wk[:, g * D:(g + 1) * D], Wv[:, g * D:(g + 1) * D]],
                axis=1)).astype(BF16_NP),
            "wo": np.ascontiguousarray(Wo[g * 256:(g + 1) * 256, :]),
        })
    res = bass_utils.run_bass_kernel_spmd(nc, in_maps, core_ids=list(range(8)))
    outs = [res.results[c]["out"] for c in range(8)]
    full = np.empty((B, N, C), np.float32)
    for b in range(B):
        acc = outs[4 * b].astype(np.float32)
        for g in range(1, 4):
            acc = acc + outs[4 * b + g]
        full[b] = acc.T + bo[None, :]
    return full
